# revision 17
# baseline (speedup 1.0000x reference)
"""Trainium2 Bass kernel for nn_CrossAttention_79448305041860.

Dual cross-attention (q1, q2 vs shared kv) + concat + out-proj + LayerNorm,
B=4, E=256, N=64*64=4096 tokens.

Sharding: 8 cores = 4 batches x 2 query-token halves. Each core computes
K,V for its batch (replicated across the pair of cores sharing a batch) and
the full pipeline for its 2048-query-token slice. No cross-core comm.

v2 vs baseline:
  - All attention matmul operands are bf16 (inputs cast host-side, halving
    input DMA): same PE rate as fp32r but FWL weight loads (~2x faster,
    fully hidden) remove the ~9% LDWEIGHTS stall seen in the fp32r trace.
  - exp runs on [P, 1024] psum tiles (2 banks per S group) to amortize the
    ~352-cycle ACT fixed cost: 1.12 ns/elem vs 1.41 at 512.
  - Softmax-denominator accumulation in bf16 (2x DVE mode).
  - Phase 2 (out-proj + LN + transpose + store) is interleaved into the
    set-2 attention windows, one q-block behind, so its PE work fills the
    o_ps-drain bubbles and its DVE/ACT work hides under attention matmuls.
  - rstd = 1/sqrt(var+eps) via quake-rsqrt on DVE (bitcast + 2 Newton
    steps): keeps Sqrt out of the ACT queue, whose table set would thrash
    against exp (no ACT table set contains both).
"""

import numpy as np
from contextlib import ExitStack

import ml_dtypes

import concourse.bass as bass
import concourse.mybir as mybir
import concourse.tile as tile
from concourse import bacc
from concourse.masks import make_identity

FP32 = mybir.dt.float32
BF16 = mybir.dt.bfloat16
I32 = mybir.dt.int32
AF = mybir.ActivationFunctionType
ALU = mybir.AluOpType

P = 128
B = 4
E = 256            # embed dim
ET = E // P        # 2 e-tiles
CKV = 512          # kv channels
CT = CKV // P      # 4 c-tiles
CQ = 256           # q channels
CQT = CQ // P      # 2 c-tiles
N = 4096           # kv tokens per batch
NKT = N // P       # 32 k token-tiles
NK2 = NKT // 2     # 16 k-tile pairs
NQ = 2048          # query tokens per core
QB = 512           # q block (psum bank width)
NQB = NQ // QB     # 4 q blocks
NT = NQ // P       # 16 token-tiles per core
SCALE = 1.0 / 16.0  # 1/sqrt(E)
LN_EPS = 1e-5
QUAKE = 0x5F3759DF


def _bcast_row(nc, dram_handle, sbuf_tile):
    """DMA-broadcast a [E] dram vector to all partitions of a [P, E] tile."""
    src_ap = dram_handle[:]
    bcast = bass.AP(
        tensor=src_ap.tensor,
        offset=src_ap.offset,
        ap=[[0, P], *src_ap.ap],
    )
    nc.gpsimd.dma_start(out=sbuf_tile[:], in_=bcast)


def build_nc():
    nc = bacc.Bacc()

    xq1_d = nc.dram_tensor("xq1", [CQ, NQ], BF16, kind="ExternalInput")
    xq2_d = nc.dram_tensor("xq2", [CQ, NQ], BF16, kind="ExternalInput")
    xkv_d = nc.dram_tensor("xkv", [CKV, N], BF16, kind="ExternalInput")
    wq1t_d = nc.dram_tensor("wq1t", [CQ, E], BF16, kind="ExternalInput")
    wq2t_d = nc.dram_tensor("wq2t", [CQ, E], BF16, kind="ExternalInput")
    wkt_d = nc.dram_tensor("wkt", [CKV, E], BF16, kind="ExternalInput")
    wvt_d = nc.dram_tensor("wvt", [CKV, E], BF16, kind="ExternalInput")
    wo1t_d = nc.dram_tensor("wo1t", [E, E], BF16, kind="ExternalInput")
    wo2t_d = nc.dram_tensor("wo2t", [E, E], BF16, kind="ExternalInput")
    bq1_d = nc.dram_tensor("bq1", [E], FP32, kind="ExternalInput")
    bq2_d = nc.dram_tensor("bq2", [E], FP32, kind="ExternalInput")
    bk_d = nc.dram_tensor("bk", [E], FP32, kind="ExternalInput")
    bv_d = nc.dram_tensor("bv", [E], FP32, kind="ExternalInput")
    bo_d = nc.dram_tensor("bo", [E], FP32, kind="ExternalInput")
    lnw_d = nc.dram_tensor("lnw", [E], FP32, kind="ExternalInput")
    lnb_d = nc.dram_tensor("lnb", [E], FP32, kind="ExternalInput")
    out_d = nc.dram_tensor("out", [E, NQ], FP32, kind="ExternalOutput")

    with tile.TileContext(nc) as tc, ExitStack() as ctx:
        const = ctx.enter_context(tc.tile_pool(name="const", bufs=1))
        wts = ctx.enter_context(tc.tile_pool(name="wts", bufs=1))
        bigin = ctx.enter_context(tc.tile_pool(name="bigin", bufs=1))
        keep = ctx.enter_context(tc.tile_pool(name="keep", bufs=1))
        flow = ctx.enter_context(tc.tile_pool(name="flow", bufs=1))
        ps_s = ctx.enter_context(tc.tile_pool(name="ps_s", bufs=2, space="PSUM"))
        ps_o = ctx.enter_context(tc.tile_pool(name="ps_o", bufs=2, space="PSUM"))
        ps_c = ctx.enter_context(tc.tile_pool(name="ps_c", bufs=2, space="PSUM"))

        # ---- weights / biases ----
        # The kv-path weights ride the two HWDGE queues ahead of the first
        # kv chunks so the first projection matmuls start ~2us earlier;
        # everything later goes on gpsimd (SWDGE).
        def _load_w(name, dram, ctiles, eng):
            t = wts.tile([P, ctiles, E], BF16, name=name)
            eng.dma_start(t[:], dram[:].rearrange("(o p) e -> p o e", p=P))
            return t

        wkt = _load_w("wkt", wkt_d, CT, nc.sync)
        wvt = _load_w("wvt", wvt_d, CT, nc.scalar)
        bk = wts.tile([P, ET], FP32, name="bk")
        nc.gpsimd.dma_start(bk[:], bk_d[:].rearrange("(o p) -> p o", p=P))
        bv_b = wts.tile([P, E], FP32, name="bv_b")
        _bcast_row(nc, bv_d, bv_b)

        wq1t = _load_w("wq1t", wq1t_d, CQT, nc.gpsimd)
        wq2t = _load_w("wq2t", wq2t_d, CQT, nc.gpsimd)
        bq1 = wts.tile([P, ET], FP32, name="bq1")
        nc.gpsimd.dma_start(bq1[:], bq1_d[:].rearrange("(o p) -> p o", p=P))
        bq2 = wts.tile([P, ET], FP32, name="bq2")
        nc.gpsimd.dma_start(bq2[:], bq2_d[:].rearrange("(o p) -> p o", p=P))

        wo1t = _load_w("wo1t", wo1t_d, ET, nc.gpsimd)
        wo2t = _load_w("wo2t", wo2t_d, ET, nc.gpsimd)
        bo_b = wts.tile([P, E], FP32, name="bo_b")
        _bcast_row(nc, bo_d, bo_b)
        # LN affine folded into the post-transpose ACT drain, where the
        # channel dim sits on partitions: per-partition scalar layout.
        lnw_c = wts.tile([P, ET], FP32, name="lnw_c")
        nc.gpsimd.dma_start(lnw_c[:], lnw_d[:].rearrange("(o p) -> p o", p=P))
        lnb_c = wts.tile([P, ET], FP32, name="lnb_c")
        nc.gpsimd.dma_start(lnb_c[:], lnb_d[:].rearrange("(o p) -> p o", p=P))

        # ---- constants ----
        ident = const.tile([P, P], FP32, name="ident")
        make_identity(nc, ident)
        ones = const.tile([P, 2], BF16, name="ones")
        nc.vector.memset(ones, 1.0)

        # ---- PE warmup ----
        # Dummy matmuls on the identity while the first kv chunks stream in:
        # keeps the PE_HAM activity window busy so the clock gate opens to
        # 2.4 GHz before the first real projection matmul (otherwise phase 0
        # runs at the cold 1.2 GHz for its first ~20us).
        warm_ps = ps_s.tile([P, 2, QB], FP32, name="warm_ps", tag="s")
        for _ in range(48):
            nc.tensor.matmul(
                warm_ps[:, 0, :P], ident[:], ident[:], start=True, stop=True
            )

        # ---- phase 0: K^T, V projections; Q^T streamed per window ----
        ktm = keep.tile([P, ET, N], BF16, name="ktm")    # K^T e-major
        vtm = keep.tile([P, NKT, E], BF16, name="vtm")   # V token-major

        QCH = 512
        qt1 = keep.tile([P, ET, NQ], BF16, name="qt1")   # Q1^T e-major
        qt2 = keep.tile([P, ET, NQ], BF16, name="qt2")
        q_specs = [
            (xq_d, wqt, bq, qt, ch)
            for (xq_d, wqt, bq, qt) in (
                (xq1_d, wq1t, bq1, qt1),
                (xq2_d, wq2t, bq2, qt2),
            )
            for ch in range(NQ // QCH)
        ]

        def _load_xq(spec):
            # q-input chunks ride the otherwise-idle SWDGE (gpsimd) queue so
            # the two HWDGE queues are dedicated to kv at startup.
            xq_d, _, _, _, ch = spec
            t = bigin.tile([P, CQT, QCH], BF16, name="xq", tag="xq", bufs=2)
            nc.gpsimd.dma_start(
                t[:],
                xq_d[:].rearrange("(o p) n -> p o n", p=P)[
                    :, :, ch * QCH : (ch + 1) * QCH
                ],
            )
            return t

        PREFETCH = 2
        xq_tiles = {i: _load_xq(q_specs[i]) for i in range(PREFETCH)}

        # kv streams in small chunks -- smallest first so the PE starts
        # within ~2us -- alternating between the two HWDGE queues
        # (sync / scalar) to double streaming bandwidth.
        KV_CHUNKS = [128, 128, 256, 256] + [256] * 13
        kv_off = 0
        for ci, kvch in enumerate(KV_CHUNKS):
            xkv_sb = bigin.tile([P, CT, 256], BF16, name="xkv", tag="xkv", bufs=4)
            dma_eng = nc.sync if ci % 2 == 0 else nc.scalar
            dma_eng.dma_start(
                xkv_sb[:, :, :kvch],
                xkv_d[:].rearrange("(o p) n -> p o n", p=P)[
                    :, :, kv_off : kv_off + kvch
                ],
            )
            # K^T for these token-columns
            for t in range(ET):
                for cc in range(0, kvch, QB):
                    w = min(QB, kvch - cc)
                    ps = ps_s.tile([P, 2, QB], FP32, name="kps", tag="s")
                    for j in range(CT):
                        nc.tensor.matmul(
                            ps[:, 0, :w],
                            wkt[:, j, t * P : (t + 1) * P],
                            xkv_sb[:, j, cc : cc + w],
                            start=(j == 0),
                            stop=(j == CT - 1),
                        )
                    nc.scalar.activation(
                        ktm[:, t, kv_off + cc : kv_off + cc + w],
                        ps[:, 0, :w],
                        AF.Identity,
                        bias=bk[:, t : t + 1],
                        scale=1.0,
                    )
            # V for these token-rows
            for v in range(kvch // P):
                kt_idx = (kv_off // P) + v
                ps = ps_o.tile([P, E], FP32, name="vps", tag="o")
                for j in range(CT):
                    nc.tensor.matmul(
                        ps[:],
                        xkv_sb[:, j, v * P : (v + 1) * P],
                        wvt[:, j, :],
                        start=(j == 0),
                        stop=(j == CT - 1),
                    )
                nc.vector.tensor_tensor(vtm[:, kt_idx, :], ps[:], bv_b[:], ALU.add)
            kv_off += kvch

        def _qt_proj_chunk(i):
            """Project one streamed q-input chunk into its Q^T slice."""
            xq_d, wqt, bq, qt, ch = q_specs[i]
            xq_sb = xq_tiles.pop(i)
            if i + PREFETCH < len(q_specs):
                xq_tiles[i + PREFETCH] = _load_xq(q_specs[i + PREFETCH])
            for t in range(ET):
                ps = ps_s.tile([P, 2, QB], FP32, name="qps", tag="s")
                for j in range(CQT):
                    nc.tensor.matmul(
                        ps[:, 0, :],
                        wqt[:, j, t * P : (t + 1) * P],
                        xq_sb[:, j, :],
                        start=(j == 0),
                        stop=(j == CQT - 1),
                    )
                nc.vector.tensor_scalar(
                    qt[:, t, ch * QCH : (ch + 1) * QCH],
                    ps[:, 0, :],
                    bq[:, t : t + 1],
                    None,
                    op0=ALU.add,
                )

        # ---- phases 1+2: attention, with out-proj/LN/store interleaved ----
        o1ut = keep.tile([P, ET, NQ], BF16, name="o1ut")  # unnormalized out1^T
        o2ut = keep.tile([P, ET, NQ], BF16, name="o2ut")
        r1 = keep.tile([P, NT], FP32, name="r1")          # 1/denom per token
        r2 = keep.tile([P, NT], FP32, name="r2")

        out_r = out_d[:].rearrange("(o p) n -> p o n", p=P)
        bo_bc = bo_b[:, None, :].to_broadcast([P, 2, E])

        # per-qb phase-2 state, carried one window
        state = {}

        def _attention_kloop(si, qb, qt):
            """S -> exp -> PV for one (set, q-block). The S matmuls for pair
            k2+1 are issued BEFORE the PV matmuls for pair k2 so the exp
            latency hides under PE work (engines execute their queues
            in-order)."""
            _qt_proj_chunk(si * NQB + qb)
            qsl = slice(qb * QB, (qb + 1) * QB)
            o_ps = [
                ps_o.tile([P, QB], FP32, name=f"ops{t}", tag="o")
                for t in range(ET)
            ]
            acc2 = flow.tile([P, 2, QB], BF16, name="acc2", tag="acc", bufs=2)
            s_tiles = {}

            def emit_s(k2):
                s_ps = ps_s.tile([P, 2, QB], FP32, name="sps", tag="s")
                for kk in range(2):
                    k = 2 * k2 + kk
                    for t in range(ET):
                        nc.tensor.matmul(
                            s_ps[:, kk, :],
                            ktm[:, t, k * P : (k + 1) * P],
                            qt[:, t, qsl],
                            start=(t == 0),
                            stop=(t == ET - 1),
                        )
                s_tiles[k2] = s_ps

            emit_s(0)
            emit_s(1)
            return qsl, o_ps, acc2, s_tiles

        def _attention_rest(si, qb, qt, qsl, o_ps, acc2, s_tiles, out_t):
            for k2 in range(NK2):
                s_ps = s_tiles.pop(k2)
                pt = flow.tile([P, 2, QB], BF16, name="pt", tag="pt", bufs=8)
                nc.scalar.activation(pt[:], s_ps[:], AF.Exp, scale=SCALE)
                for kk in range(2):
                    k = 2 * k2 + kk
                    for t in range(ET):
                        nc.tensor.matmul(
                            o_ps[t][:],
                            vtm[:, k, t * P : (t + 1) * P],
                            pt[:, kk, :],
                            start=(k2 == 0 and kk == 0),
                            stop=(k2 == NK2 - 1 and kk == 1),
                        )
                if k2 + 2 < NK2:
                    # keep the S stream one pair ahead of PV
                    s_ps2 = ps_s.tile([P, 2, QB], FP32, name="sps", tag="s")
                    for kk in range(2):
                        k = 2 * (k2 + 2) + kk
                        for t in range(ET):
                            nc.tensor.matmul(
                                s_ps2[:, kk, :],
                                ktm[:, t, k * P : (k + 1) * P],
                                qt[:, t, qsl],
                                start=(t == 0),
                                stop=(t == ET - 1),
                            )
                    s_tiles[k2 + 2] = s_ps2
                if k2 == 0:
                    nc.vector.tensor_copy(acc2[:], pt[:])
                else:
                    nc.vector.tensor_tensor(acc2[:], acc2[:], pt[:], ALU.add)
            # out^T psum drains (gate the next window's PV via o_ps rotation)
            # and the bf16 accumulator merge for the denominators.
            for t in range(ET):
                nc.vector.tensor_copy(out_t[:, t, qsl], o_ps[t][:])
            acc = flow.tile([P, QB], BF16, name="acc", tag="accm", bufs=2)
            nc.vector.tensor_tensor(acc[:], acc2[:, 0, :], acc2[:, 1, :], ALU.add)
            return acc

        def _attention_denom(qb, acc, r_t):
            """Denominator matmuls + reciprocal; for set-1 this is deferred
            into the NEXT window's head so the PE never waits on the DVE
            accumulator merge."""
            d_ps = ps_c.tile([P, QB // P, 2], FP32, name="dps", tag="c")
            for i in range(QB // P):
                nc.tensor.matmul(
                    d_ps[:, i, :],
                    acc[:, i * P : (i + 1) * P],
                    ones[:],
                    start=True,
                    stop=True,
                )
            nc.vector.reciprocal(
                r_t[:, qb * (QB // P) : (qb + 1) * (QB // P)], d_ps[:, :, 0]
            )

        def _quake_rstd(var_ap, n):
            """rstd = 1/sqrt(var+eps) on DVE: quake initial guess + 2 Newton
            steps (keeps Sqrt out of the ACT queue -- table-set thrash)."""
            vr = flow.tile([P, n], FP32, name="vr", tag="vr", bufs=2)
            yi = flow.tile([P, n], I32, name="yi", tag="yi", bufs=2)
            t1 = flow.tile([P, n], FP32, name="t1", tag="t1", bufs=2)
            rstd = flow.tile([P, n], FP32, name="rstd", tag="rstd", bufs=4)
            nc.vector.tensor_scalar(vr[:], var_ap, LN_EPS, None, op0=ALU.add)
            nc.vector.tensor_scalar(
                yi[:], vr[:].bitcast(I32), 1, None, op0=ALU.logical_shift_right
            )
            nc.vector.tensor_scalar(yi[:], yi[:], -1, None, op0=ALU.bitwise_xor)
            nc.vector.tensor_scalar(yi[:], yi[:], QUAKE + 1, None, op0=ALU.add)
            y0 = yi[:].bitcast(FP32)
            nc.vector.tensor_tensor(t1[:], y0, y0, ALU.mult)
            nc.vector.tensor_tensor(t1[:], t1[:], vr[:], ALU.mult)
            nc.vector.tensor_scalar(t1[:], t1[:], -0.5, 1.5, op0=ALU.mult, op1=ALU.add)
            nc.vector.tensor_tensor(rstd[:], y0, t1[:], ALU.mult)
            nc.vector.tensor_tensor(t1[:], rstd[:], rstd[:], ALU.mult)
            nc.vector.tensor_tensor(t1[:], t1[:], vr[:], ALU.mult)
            nc.vector.tensor_scalar(t1[:], t1[:], -0.5, 1.5, op0=ALU.mult, op1=ALU.add)
            nc.vector.tensor_tensor(rstd[:], rstd[:], t1[:], ALU.mult)
            return rstd

        def _outproj_pair(qb, pr, y):
            """Out-proj + softmax-normalize + combine for one token-pair."""
            for h in range(2):
                nt = qb * 4 + pr * 2 + h
                nsl = slice(nt * P, (nt + 1) * P)
                yp = ps_c.tile([P, 2, E], FP32, name="yp", tag="c")
                for j in range(ET):
                    nc.tensor.matmul(
                        yp[:, 0, :],
                        o1ut[:, j, nsl],
                        wo1t[:, j, :],
                        start=(j == 0),
                        stop=(j == ET - 1),
                    )
                for j in range(ET):
                    nc.tensor.matmul(
                        yp[:, 1, :],
                        o2ut[:, j, nsl],
                        wo2t[:, j, :],
                        start=(j == 0),
                        stop=(j == ET - 1),
                    )
                # y = y1*r1 (ACT) ; y += y2*r2 (DVE, fused)
                nc.scalar.activation(
                    y[:, h, :], yp[:, 0, :], AF.Identity,
                    scale=r1[:, nt : nt + 1],
                )
                nc.vector.scalar_tensor_tensor(
                    y[:, h, :], yp[:, 1, :], r2[:, nt : nt + 1], y[:, h, :],
                    op0=ALU.mult, op1=ALU.add,
                )
            nc.vector.tensor_tensor(y[:], y[:], bo_bc, ALU.add)

        def _transpose_store_tile(nt, y_h):
            """Transpose one token-tile to channel-major, LN-affine, store."""
            tp = ps_c.tile([P, ET, P], FP32, name="tp", tag="c")
            yt = flow.tile([P, ET, P], FP32, name="yt", tag="yt", bufs=3)
            for t in range(ET):
                nc.tensor.transpose(
                    tp[:, t, :], y_h[:, t * P : (t + 1) * P], ident[:]
                )
            for t in range(ET):
                nc.scalar.activation(
                    yt[:, t, :], tp[:, t, :], AF.Identity,
                    bias=lnb_c[:, t : t + 1], scale=lnw_c[:, t : t + 1],
                )
            eng = nc.sync if nt % 2 == 0 else nc.scalar
            nsl = slice(nt * P, (nt + 1) * P)
            for t in range(ET):
                eng.dma_start(out_r[:, t, nsl], yt[:, t, :])

        def _phase2_front(qb):
            """Out-proj + softmax-normalize + combine + LN stats for the 4
            token-tiles of set-2 q-block qb. Emitted right after qb's
            attention window."""
            ys = []
            mv = flow.tile([P, 4, 2], FP32, name="mv", tag="mv", bufs=2)
            for pr in range(2):
                y = flow.tile([P, 2, E], FP32, name="y", tag="y", bufs=4)
                ys.append(y)
                _outproj_pair(qb, pr, y)
                for h in range(2):
                    st6 = flow.tile([P, 6], FP32, name="st6", tag="st6", bufs=3)
                    nc.vector.bn_stats(out=st6[:], in_=y[:, h, :])
                    nc.vector.bn_aggr(out=mv[:, pr * 2 + h, :], in_=st6[:])
            rstd = _quake_rstd(mv[:, :, 1], 4)
            state[qb] = (ys, mv, rstd)

        def _phase2_final(qb):
            """Pair-pipelined phase-2 for the last q-block: shortens the
            serialized dependency chain in the kernel tail."""
            for pr in range(2):
                y = flow.tile([P, 2, E], FP32, name="y", tag="y", bufs=4)
                _outproj_pair(qb, pr, y)
                mv = flow.tile([P, 2, 2], FP32, name="mvf", tag="mv2", bufs=2)
                for h in range(2):
                    st6 = flow.tile([P, 6], FP32, name="st6", tag="st6", bufs=3)
                    nc.vector.bn_stats(out=st6[:], in_=y[:, h, :])
                    nc.vector.bn_aggr(out=mv[:, h, :], in_=st6[:])
                rstd = _quake_rstd(mv[:, :, 1], 2)
                for h in range(2):
                    nc.vector.tensor_scalar(
                        y[:, h, :], y[:, h, :],
                        mv[:, h, 0:1], rstd[:, h : h + 1],
                        op0=ALU.subtract, op1=ALU.mult,
                    )
                for h in range(2):
                    _transpose_store_tile(qb * 4 + pr * 2 + h, y[:, h, :])

        def _phase2_back_dve(qb):
            """Normalize for q-block qb (window qb+1, early). The LN affine
            rides the post-transpose ACT drain instead."""
            ys, mv, rstd = state[qb]
            for pr in range(2):
                y = ys[pr]
                for h in range(2):
                    i = pr * 2 + h
                    nc.vector.tensor_scalar(
                        y[:, h, :], y[:, h, :],
                        mv[:, i, 0:1], rstd[:, i : i + 1],
                        op0=ALU.subtract, op1=ALU.mult,
                    )

        def _phase2_back_pe(qb):
            """Transpose to channel-major + affine + store for q-block qb."""
            ys, _, _ = state.pop(qb)
            for pr in range(2):
                for h in range(2):
                    _transpose_store_tile(qb * 4 + pr * 2 + h, ys[pr][:, h, :])

        # set 1: plain attention windows. The denominator matmuls of window
        # qb run inside window qb+1's head so the PE never waits on the DVE
        # accumulator merge at a window boundary.
        pend = None
        for qb in range(NQB):
            qsl, o_ps, acc2, s_tiles = _attention_kloop(0, qb, qt1)
            if pend is not None:
                _attention_denom(*pend)
            acc = _attention_rest(0, qb, qt1, qsl, o_ps, acc2, s_tiles, o1ut)
            pend = (qb, acc, r1)
        # set 2: attention + interleaved phase-2 (one q-block behind)
        for qb in range(NQB):
            if qb > 0:
                _phase2_back_dve(qb - 1)
            qsl, o_ps, acc2, s_tiles = _attention_kloop(1, qb, qt2)
            if pend is not None:
                _attention_denom(*pend)
                pend = None
            acc = _attention_rest(1, qb, qt2, qsl, o_ps, acc2, s_tiles, o2ut)
            if qb > 0:
                _phase2_back_pe(qb - 1)
            _attention_denom(qb, acc, r2)
            if qb < NQB - 1:
                _phase2_front(qb)
            else:
                _phase2_final(qb)

    nc.compile()
    return nc


_CACHE = {}


def _get_nc():
    if "nc" not in _CACHE:
        _CACHE["nc"] = build_nc()
    return _CACHE["nc"]


def make_in_maps(q1, q2, kv, wq1, bq1, wq2, bq2, wk, bk, wv, bv, wo, bo, ln_w, ln_b):
    f32 = lambda a: np.ascontiguousarray(np.asarray(a, dtype=np.float32))
    b16 = lambda a: np.ascontiguousarray(
        np.asarray(a, dtype=np.float32).astype(ml_dtypes.bfloat16)
    )
    base = {
        "wq1t": b16(np.asarray(wq1).T),
        "wq2t": b16(np.asarray(wq2).T),
        "wkt": b16(np.asarray(wk).T),
        "wvt": b16(np.asarray(wv).T),
        "wo1t": b16(np.asarray(wo)[:, :E].T),
        "wo2t": b16(np.asarray(wo)[:, E:].T),
        "bq1": f32(bq1),
        "bq2": f32(bq2),
        "bk": f32(bk),
        "bv": f32(bv),
        "bo": f32(bo),
        "lnw": f32(ln_w),
        "lnb": f32(ln_b),
    }
    q1 = np.asarray(q1)
    q2 = np.asarray(q2)
    kv_flat = [b16(np.asarray(kv)[b].reshape(CKV, N)) for b in range(B)]
    in_maps = []
    for c in range(8):
        b, h = divmod(c, 2)
        m = dict(base)
        m["xq1"] = b16(q1[b, :, h * 32 : (h + 1) * 32, :].reshape(CQ, NQ))
        m["xq2"] = b16(q2[b, :, h * 32 : (h + 1) * 32, :].reshape(CQ, NQ))
        m["xkv"] = kv_flat[b]
        in_maps.append(m)
    return in_maps


def assemble_output(results):
    out = np.empty((B, E, 64, 64), dtype=np.float32)
    for c in range(8):
        b, h = divmod(c, 2)
        out[b, :, h * 32 : (h + 1) * 32, :] = results[c]["out"].reshape(E, 32, 64)
    return out


def kernel(**inputs):
    from concourse.bass_utils import run_bass_kernel_spmd

    nc = _get_nc()
    in_maps = make_in_maps(**inputs)
    res = run_bass_kernel_spmd(nc, in_maps, list(range(8)))
    return assemble_output(res.results)


if __name__ == "__main__":
    nc = build_nc()
    print("built ok")


# revision 22
# speedup vs baseline: 1.0358x; 1.0358x over previous
"""Trainium2 Bass kernel for nn_CrossAttention_79448305041860.

Dual cross-attention (q1, q2 vs shared kv) + concat + out-proj + LayerNorm,
B=4, E=256, N=64*64=4096 tokens.

Sharding: 8 cores = 4 batches x 2 query-token halves. Each core computes
K,V for its batch (replicated across the pair of cores sharing a batch) and
the full pipeline for its 2048-query-token slice. No cross-core comm.

v2 vs baseline:
  - All attention matmul operands are bf16 (inputs cast host-side, halving
    input DMA): same PE rate as fp32r but FWL weight loads (~2x faster,
    fully hidden) remove the ~9% LDWEIGHTS stall seen in the fp32r trace.
  - exp runs on [P, 1024] psum tiles (2 banks per S group) to amortize the
    ~352-cycle ACT fixed cost: 1.12 ns/elem vs 1.41 at 512.
  - Softmax-denominator accumulation in bf16 (2x DVE mode).
  - Phase 2 (out-proj + LN + transpose + store) is interleaved into the
    set-2 attention windows, one q-block behind, so its PE work fills the
    o_ps-drain bubbles and its DVE/ACT work hides under attention matmuls.
  - rstd = 1/sqrt(var+eps) via quake-rsqrt on DVE (bitcast + 2 Newton
    steps): keeps Sqrt out of the ACT queue, whose table set would thrash
    against exp (no ACT table set contains both).
"""

import numpy as np
from contextlib import ExitStack

import ml_dtypes

import concourse.bass as bass
import concourse.mybir as mybir
import concourse.tile as tile
from concourse import bacc
from concourse.masks import make_identity

FP32 = mybir.dt.float32
BF16 = mybir.dt.bfloat16
I32 = mybir.dt.int32
AF = mybir.ActivationFunctionType
ALU = mybir.AluOpType

P = 128
B = 4
E = 256            # embed dim
ET = E // P        # 2 e-tiles
CKV = 512          # kv channels
CT = CKV // P      # 4 c-tiles
CQ = 256           # q channels
CQT = CQ // P      # 2 c-tiles
N = 4096           # kv tokens per batch
NKT = N // P       # 32 k token-tiles
NK2 = NKT // 2     # 16 k-tile pairs
NQ = 2048          # query tokens per core
QB = 512           # q block (psum bank width)
NQB = NQ // QB     # 4 q blocks
NT = NQ // P       # 16 token-tiles per core
SCALE = 1.0 / 16.0  # 1/sqrt(E)
LN_EPS = 1e-5
QUAKE = 0x5F3759DF


def _bcast_row(nc, dram_handle, sbuf_tile):
    """DMA-broadcast a [E] dram vector to all partitions of a [P, E] tile."""
    src_ap = dram_handle[:]
    bcast = bass.AP(
        tensor=src_ap.tensor,
        offset=src_ap.offset,
        ap=[[0, P], *src_ap.ap],
    )
    nc.gpsimd.dma_start(out=sbuf_tile[:], in_=bcast)


def build_nc():
    nc = bacc.Bacc()

    xq1_d = nc.dram_tensor("xq1", [CQ, NQ], BF16, kind="ExternalInput")
    xq2_d = nc.dram_tensor("xq2", [CQ, NQ], BF16, kind="ExternalInput")
    xkv_d = nc.dram_tensor("xkv", [CKV, N], BF16, kind="ExternalInput")
    wq1t_d = nc.dram_tensor("wq1t", [CQ, E], BF16, kind="ExternalInput")
    wq2t_d = nc.dram_tensor("wq2t", [CQ, E], BF16, kind="ExternalInput")
    wkt_d = nc.dram_tensor("wkt", [CKV, E], BF16, kind="ExternalInput")
    wvt_d = nc.dram_tensor("wvt", [CKV, E], BF16, kind="ExternalInput")
    wo1t_d = nc.dram_tensor("wo1t", [E, E], BF16, kind="ExternalInput")
    wo2t_d = nc.dram_tensor("wo2t", [E, E], BF16, kind="ExternalInput")
    bq1_d = nc.dram_tensor("bq1", [E], FP32, kind="ExternalInput")
    bq2_d = nc.dram_tensor("bq2", [E], FP32, kind="ExternalInput")
    bk_d = nc.dram_tensor("bk", [E], FP32, kind="ExternalInput")
    bv_d = nc.dram_tensor("bv", [E], FP32, kind="ExternalInput")
    bo_d = nc.dram_tensor("bo", [E], FP32, kind="ExternalInput")
    lnw_d = nc.dram_tensor("lnw", [E], FP32, kind="ExternalInput")
    lnb_d = nc.dram_tensor("lnb", [E], FP32, kind="ExternalInput")
    out_d = nc.dram_tensor("out", [E, NQ], FP32, kind="ExternalOutput")

    with tile.TileContext(nc) as tc, ExitStack() as ctx:
        const = ctx.enter_context(tc.tile_pool(name="const", bufs=1))
        wts = ctx.enter_context(tc.tile_pool(name="wts", bufs=1))
        bigin = ctx.enter_context(tc.tile_pool(name="bigin", bufs=1))
        keep = ctx.enter_context(tc.tile_pool(name="keep", bufs=1))
        flow = ctx.enter_context(tc.tile_pool(name="flow", bufs=1))
        ps_s = ctx.enter_context(tc.tile_pool(name="ps_s", bufs=2, space="PSUM"))
        ps_o = ctx.enter_context(tc.tile_pool(name="ps_o", bufs=2, space="PSUM"))
        ps_c = ctx.enter_context(tc.tile_pool(name="ps_c", bufs=2, space="PSUM"))

        # ---- weights / biases ----
        # The kv-path weights ride the two HWDGE queues ahead of the first
        # kv chunks so the first projection matmuls start ~2us earlier;
        # everything later goes on gpsimd (SWDGE).
        def _load_w(name, dram, ctiles, eng):
            t = wts.tile([P, ctiles, E], BF16, name=name)
            eng.dma_start(t[:], dram[:].rearrange("(o p) e -> p o e", p=P))
            return t

        wkt = _load_w("wkt", wkt_d, CT, nc.sync)
        wvt = _load_w("wvt", wvt_d, CT, nc.scalar)
        bk = wts.tile([P, ET], FP32, name="bk")
        nc.gpsimd.dma_start(bk[:], bk_d[:].rearrange("(o p) -> p o", p=P))
        bv_b = wts.tile([P, E], FP32, name="bv_b")
        _bcast_row(nc, bv_d, bv_b)

        wq1t = _load_w("wq1t", wq1t_d, CQT, nc.gpsimd)
        wq2t = _load_w("wq2t", wq2t_d, CQT, nc.gpsimd)
        bq1 = wts.tile([P, ET], FP32, name="bq1")
        nc.gpsimd.dma_start(bq1[:], bq1_d[:].rearrange("(o p) -> p o", p=P))
        bq2 = wts.tile([P, ET], FP32, name="bq2")
        nc.gpsimd.dma_start(bq2[:], bq2_d[:].rearrange("(o p) -> p o", p=P))

        wo1t = _load_w("wo1t", wo1t_d, ET, nc.gpsimd)
        wo2t = _load_w("wo2t", wo2t_d, ET, nc.gpsimd)
        bo_b = wts.tile([P, E], FP32, name="bo_b")
        _bcast_row(nc, bo_d, bo_b)
        # LN affine folded into the post-transpose ACT drain, where the
        # channel dim sits on partitions: per-partition scalar layout.
        lnw_c = wts.tile([P, ET], FP32, name="lnw_c")
        nc.gpsimd.dma_start(lnw_c[:], lnw_d[:].rearrange("(o p) -> p o", p=P))
        lnb_c = wts.tile([P, ET], FP32, name="lnb_c")
        nc.gpsimd.dma_start(lnb_c[:], lnb_d[:].rearrange("(o p) -> p o", p=P))

        # ---- constants ----
        ident = const.tile([P, P], FP32, name="ident")
        make_identity(nc, ident)
        ones = const.tile([P, 2], BF16, name="ones")
        nc.vector.memset(ones, 1.0)

        # ---- PE warmup ----
        # Dummy matmuls on the identity while the first kv chunks stream in:
        # keeps the PE_HAM activity window busy so the clock gate opens to
        # 2.4 GHz before the first real projection matmul (otherwise phase 0
        # runs at the cold 1.2 GHz for its first ~20us).
        warm_ps = ps_s.tile([P, 2, QB], FP32, name="warm_ps", tag="s")
        for _ in range(48):
            nc.tensor.matmul(
                warm_ps[:, 0, :P], ident[:], ident[:], start=True, stop=True
            )

        # ---- phase 0: K^T, V projections; Q^T streamed per window ----
        ktm = keep.tile([P, ET, N], BF16, name="ktm")    # K^T e-major
        vtm = keep.tile([P, NKT, E], BF16, name="vtm")   # V token-major

        QCH = 512
        qt1 = keep.tile([P, ET, NQ], BF16, name="qt1")   # Q1^T e-major
        qt2 = keep.tile([P, ET, NQ], BF16, name="qt2")
        q_specs = [
            (xq_d, wqt, bq, qt, ch)
            for (xq_d, wqt, bq, qt) in (
                (xq1_d, wq1t, bq1, qt1),
                (xq2_d, wq2t, bq2, qt2),
            )
            for ch in range(NQ // QCH)
        ]

        def _load_xq(spec):
            # q-input chunks ride the otherwise-idle SWDGE (gpsimd) queue so
            # the two HWDGE queues are dedicated to kv at startup.
            xq_d, _, _, _, ch = spec
            t = bigin.tile([P, CQT, QCH], BF16, name="xq", tag="xq", bufs=2)
            nc.gpsimd.dma_start(
                t[:],
                xq_d[:].rearrange("(o p) n -> p o n", p=P)[
                    :, :, ch * QCH : (ch + 1) * QCH
                ],
            )
            return t

        PREFETCH = 2
        xq_tiles = {i: _load_xq(q_specs[i]) for i in range(PREFETCH)}

        # kv streams in small chunks -- smallest first so the PE starts
        # within ~2us -- alternating between the two HWDGE queues
        # (sync / scalar) to double streaming bandwidth.
        KV_CHUNKS = [128, 128, 256, 256] + [256] * 13
        kv_off = 0
        for ci, kvch in enumerate(KV_CHUNKS):
            xkv_sb = bigin.tile([P, CT, 256], BF16, name="xkv", tag="xkv", bufs=4)
            dma_eng = nc.sync if ci % 2 == 0 else nc.scalar
            dma_eng.dma_start(
                xkv_sb[:, :, :kvch],
                xkv_d[:].rearrange("(o p) n -> p o n", p=P)[
                    :, :, kv_off : kv_off + kvch
                ],
            )
            # K^T for these token-columns
            for t in range(ET):
                for cc in range(0, kvch, QB):
                    w = min(QB, kvch - cc)
                    ps = ps_s.tile([P, 2, QB], FP32, name="kps", tag="s")
                    for j in range(CT):
                        nc.tensor.matmul(
                            ps[:, 0, :w],
                            wkt[:, j, t * P : (t + 1) * P],
                            xkv_sb[:, j, cc : cc + w],
                            start=(j == 0),
                            stop=(j == CT - 1),
                        )
                    nc.scalar.activation(
                        ktm[:, t, kv_off + cc : kv_off + cc + w],
                        ps[:, 0, :w],
                        AF.Identity,
                        bias=bk[:, t : t + 1],
                        scale=1.0,
                    )
            # V for these token-rows
            for v in range(kvch // P):
                kt_idx = (kv_off // P) + v
                ps = ps_o.tile([P, E], FP32, name="vps", tag="o")
                for j in range(CT):
                    nc.tensor.matmul(
                        ps[:],
                        xkv_sb[:, j, v * P : (v + 1) * P],
                        wvt[:, j, :],
                        start=(j == 0),
                        stop=(j == CT - 1),
                    )
                nc.vector.tensor_tensor(vtm[:, kt_idx, :], ps[:], bv_b[:], ALU.add)
            kv_off += kvch

        def _qt_proj_chunk(i):
            """Project one streamed q-input chunk into its Q^T slice."""
            xq_d, wqt, bq, qt, ch = q_specs[i]
            xq_sb = xq_tiles.pop(i)
            if i + PREFETCH < len(q_specs):
                xq_tiles[i + PREFETCH] = _load_xq(q_specs[i + PREFETCH])
            for t in range(ET):
                ps = ps_s.tile([P, 2, QB], FP32, name="qps", tag="s")
                for j in range(CQT):
                    nc.tensor.matmul(
                        ps[:, 0, :],
                        wqt[:, j, t * P : (t + 1) * P],
                        xq_sb[:, j, :],
                        start=(j == 0),
                        stop=(j == CQT - 1),
                    )
                nc.vector.tensor_scalar(
                    qt[:, t, ch * QCH : (ch + 1) * QCH],
                    ps[:, 0, :],
                    bq[:, t : t + 1],
                    None,
                    op0=ALU.add,
                )

        # ---- phases 1+2: attention, with out-proj/LN/store interleaved ----
        o1ut = keep.tile([P, ET, NQ], BF16, name="o1ut")  # unnormalized out1^T
        o2ut = keep.tile([P, ET, NQ], BF16, name="o2ut")
        r1 = keep.tile([P, NT], FP32, name="r1")          # 1/denom per token
        r2 = keep.tile([P, NT], FP32, name="r2")

        out_r = out_d[:].rearrange("(o p) n -> p o n", p=P)
        bo_bc = bo_b[:, None, :].to_broadcast([P, 2, E])

        # per-qb phase-2 state, carried one window
        state = {}

        def _attention_kloop(si, qb, qt):
            """S -> exp -> PV for one (set, q-block). The S matmuls for pair
            k2+1 are issued BEFORE the PV matmuls for pair k2 so the exp
            latency hides under PE work (engines execute their queues
            in-order)."""
            if si == 0 and qb == 0:
                # later windows' q-projections are hoisted into the middle
                # of the previous window (see _attention_rest)
                _qt_proj_chunk(0)
            qsl = slice(qb * QB, (qb + 1) * QB)
            o_ps = [
                ps_o.tile([P, QB], FP32, name=f"ops{t}", tag="o")
                for t in range(ET)
            ]
            acc2 = flow.tile([P, 2, QB], BF16, name="acc2", tag="acc", bufs=2)
            s_tiles = {}

            def emit_s(k2):
                s_ps = ps_s.tile([P, 2, QB], FP32, name="sps", tag="s")
                for kk in range(2):
                    k = 2 * k2 + kk
                    for t in range(ET):
                        nc.tensor.matmul(
                            s_ps[:, kk, :],
                            ktm[:, t, k * P : (k + 1) * P],
                            qt[:, t, qsl],
                            start=(t == 0),
                            stop=(t == ET - 1),
                        )
                s_tiles[k2] = s_ps

            emit_s(0)
            emit_s(1)
            return qsl, o_ps, acc2, s_tiles

        def _attention_rest(si, qb, qt, qsl, o_ps, acc2, s_tiles, out_t):
            w = si * NQB + qb
            for k2 in range(NK2):
                if k2 == NK2 // 2 and w + 1 < 2 * NQB:
                    # hoist the NEXT window's q-projection to mid-window so
                    # its psum drain is long done before that window's first
                    # S matmul (removes a DVE wait at every boundary)
                    _qt_proj_chunk(w + 1)
                s_ps = s_tiles.pop(k2)
                pt = flow.tile([P, 2, QB], BF16, name="pt", tag="pt", bufs=8)
                nc.scalar.activation(pt[:], s_ps[:], AF.Exp, scale=SCALE)
                for kk in range(2):
                    k = 2 * k2 + kk
                    for t in range(ET):
                        nc.tensor.matmul(
                            o_ps[t][:],
                            vtm[:, k, t * P : (t + 1) * P],
                            pt[:, kk, :],
                            start=(k2 == 0 and kk == 0),
                            stop=(k2 == NK2 - 1 and kk == 1),
                        )
                if k2 + 2 < NK2:
                    # keep the S stream one pair ahead of PV
                    s_ps2 = ps_s.tile([P, 2, QB], FP32, name="sps", tag="s")
                    for kk in range(2):
                        k = 2 * (k2 + 2) + kk
                        for t in range(ET):
                            nc.tensor.matmul(
                                s_ps2[:, kk, :],
                                ktm[:, t, k * P : (k + 1) * P],
                                qt[:, t, qsl],
                                start=(t == 0),
                                stop=(t == ET - 1),
                            )
                    s_tiles[k2 + 2] = s_ps2
                if k2 == 0:
                    nc.vector.tensor_copy(acc2[:], pt[:])
                else:
                    nc.vector.tensor_tensor(acc2[:], acc2[:], pt[:], ALU.add)
            # out^T psum drains (gate the next window's PV via o_ps rotation)
            # and the bf16 accumulator merge for the denominators.
            for t in range(ET):
                nc.vector.tensor_copy(out_t[:, t, qsl], o_ps[t][:])
            acc = flow.tile([P, QB], BF16, name="acc", tag="accm", bufs=2)
            nc.vector.tensor_tensor(acc[:], acc2[:, 0, :], acc2[:, 1, :], ALU.add)
            return acc

        def _attention_denom(qb, acc, r_t):
            """Denominator matmuls + reciprocal; for set-1 this is deferred
            into the NEXT window's head so the PE never waits on the DVE
            accumulator merge."""
            d_ps = ps_c.tile([P, QB // P, 2], FP32, name="dps", tag="c")
            for i in range(QB // P):
                nc.tensor.matmul(
                    d_ps[:, i, :],
                    acc[:, i * P : (i + 1) * P],
                    ones[:],
                    start=True,
                    stop=True,
                )
            nc.vector.reciprocal(
                r_t[:, qb * (QB // P) : (qb + 1) * (QB // P)], d_ps[:, :, 0]
            )

        def _quake_rstd(var_ap, n):
            """rstd = 1/sqrt(var+eps) on DVE: quake initial guess + 2 Newton
            steps (keeps Sqrt out of the ACT queue -- table-set thrash)."""
            vr = flow.tile([P, n], FP32, name="vr", tag="vr", bufs=2)
            yi = flow.tile([P, n], I32, name="yi", tag="yi", bufs=2)
            t1 = flow.tile([P, n], FP32, name="t1", tag="t1", bufs=2)
            rstd = flow.tile([P, n], FP32, name="rstd", tag="rstd", bufs=4)
            nc.vector.tensor_scalar(vr[:], var_ap, LN_EPS, None, op0=ALU.add)
            nc.vector.tensor_scalar(
                yi[:], vr[:].bitcast(I32), 1, None, op0=ALU.logical_shift_right
            )
            nc.vector.tensor_scalar(yi[:], yi[:], -1, None, op0=ALU.bitwise_xor)
            nc.vector.tensor_scalar(yi[:], yi[:], QUAKE + 1, None, op0=ALU.add)
            y0 = yi[:].bitcast(FP32)
            nc.vector.tensor_tensor(t1[:], y0, y0, ALU.mult)
            nc.vector.tensor_tensor(t1[:], t1[:], vr[:], ALU.mult)
            nc.vector.tensor_scalar(t1[:], t1[:], -0.5, 1.5, op0=ALU.mult, op1=ALU.add)
            nc.vector.tensor_tensor(rstd[:], y0, t1[:], ALU.mult)
            nc.vector.tensor_tensor(t1[:], rstd[:], rstd[:], ALU.mult)
            nc.vector.tensor_tensor(t1[:], t1[:], vr[:], ALU.mult)
            nc.vector.tensor_scalar(t1[:], t1[:], -0.5, 1.5, op0=ALU.mult, op1=ALU.add)
            nc.vector.tensor_tensor(rstd[:], rstd[:], t1[:], ALU.mult)
            return rstd

        def _outproj_pair(qb, pr, y):
            """Out-proj + softmax-normalize + combine for one token-pair."""
            for h in range(2):
                nt = qb * 4 + pr * 2 + h
                nsl = slice(nt * P, (nt + 1) * P)
                yp = ps_c.tile([P, 2, E], FP32, name="yp", tag="c")
                for j in range(ET):
                    nc.tensor.matmul(
                        yp[:, 0, :],
                        o1ut[:, j, nsl],
                        wo1t[:, j, :],
                        start=(j == 0),
                        stop=(j == ET - 1),
                    )
                for j in range(ET):
                    nc.tensor.matmul(
                        yp[:, 1, :],
                        o2ut[:, j, nsl],
                        wo2t[:, j, :],
                        start=(j == 0),
                        stop=(j == ET - 1),
                    )
                # y = y1*r1 (ACT) ; y += y2*r2 (DVE, fused)
                nc.scalar.activation(
                    y[:, h, :], yp[:, 0, :], AF.Identity,
                    scale=r1[:, nt : nt + 1],
                )
                nc.vector.scalar_tensor_tensor(
                    y[:, h, :], yp[:, 1, :], r2[:, nt : nt + 1], y[:, h, :],
                    op0=ALU.mult, op1=ALU.add,
                )
            nc.vector.tensor_tensor(y[:], y[:], bo_bc, ALU.add)

        def _transpose_store_tile(nt, y_h):
            """Transpose one token-tile to channel-major, LN-affine, store."""
            tp = ps_c.tile([P, ET, P], FP32, name="tp", tag="c")
            yt = flow.tile([P, ET, P], FP32, name="yt", tag="yt", bufs=3)
            for t in range(ET):
                nc.tensor.transpose(
                    tp[:, t, :], y_h[:, t * P : (t + 1) * P], ident[:]
                )
            for t in range(ET):
                nc.scalar.activation(
                    yt[:, t, :], tp[:, t, :], AF.Identity,
                    bias=lnb_c[:, t : t + 1], scale=lnw_c[:, t : t + 1],
                )
            nsl = slice(nt * P, (nt + 1) * P)
            for t in range(ET):
                nc.sync.dma_start(out_r[:, t, nsl], yt[:, t, :])

        def _phase2_front(qb):
            """Out-proj + softmax-normalize + combine + LN stats for the 4
            token-tiles of set-2 q-block qb. Emitted right after qb's
            attention window."""
            ys = []
            mv = flow.tile([P, 4, 2], FP32, name="mv", tag="mv", bufs=2)
            for pr in range(2):
                y = flow.tile([P, 2, E], FP32, name="y", tag="y", bufs=4)
                ys.append(y)
                _outproj_pair(qb, pr, y)
                for h in range(2):
                    st6 = flow.tile([P, 6], FP32, name="st6", tag="st6", bufs=3)
                    nc.vector.bn_stats(out=st6[:], in_=y[:, h, :])
                    nc.vector.bn_aggr(out=mv[:, pr * 2 + h, :], in_=st6[:])
            rstd = _quake_rstd(mv[:, :, 1], 4)
            state[qb] = (ys, mv, rstd)

        def _phase2_back_dve(qb):
            """Normalize for q-block qb (window qb+1, early). The LN affine
            rides the post-transpose ACT drain instead."""
            ys, mv, rstd = state[qb]
            for pr in range(2):
                y = ys[pr]
                for h in range(2):
                    i = pr * 2 + h
                    nc.vector.tensor_scalar(
                        y[:, h, :], y[:, h, :],
                        mv[:, i, 0:1], rstd[:, i : i + 1],
                        op0=ALU.subtract, op1=ALU.mult,
                    )

        def _phase2_back_pe(qb):
            """Transpose to channel-major + affine + store for q-block qb."""
            ys, _, _ = state.pop(qb)
            for pr in range(2):
                for h in range(2):
                    _transpose_store_tile(qb * 4 + pr * 2 + h, ys[pr][:, h, :])

        # set 1: plain attention windows. The denominator matmuls of window
        # qb run inside window qb+1's head so the PE never waits on the DVE
        # accumulator merge at a window boundary.
        pend = None
        for qb in range(NQB):
            qsl, o_ps, acc2, s_tiles = _attention_kloop(0, qb, qt1)
            if pend is not None:
                _attention_denom(*pend)
            acc = _attention_rest(0, qb, qt1, qsl, o_ps, acc2, s_tiles, o1ut)
            pend = (qb, acc, r1)
        # set 2: attention + interleaved phase-2 (one q-block behind)
        for qb in range(NQB):
            if qb > 0:
                _phase2_back_dve(qb - 1)
            qsl, o_ps, acc2, s_tiles = _attention_kloop(1, qb, qt2)
            if pend is not None:
                _attention_denom(*pend)
                pend = None
            acc = _attention_rest(1, qb, qt2, qsl, o_ps, acc2, s_tiles, o2ut)
            if qb > 0:
                _phase2_back_pe(qb - 1)
            _attention_denom(qb, acc, r2)
            _phase2_front(qb)
        _phase2_back_dve(NQB - 1)
        _phase2_back_pe(NQB - 1)

    nc.compile()
    return nc


_CACHE = {}


def _get_nc():
    if "nc" not in _CACHE:
        _CACHE["nc"] = build_nc()
    return _CACHE["nc"]


def make_in_maps(q1, q2, kv, wq1, bq1, wq2, bq2, wk, bk, wv, bv, wo, bo, ln_w, ln_b):
    f32 = lambda a: np.ascontiguousarray(np.asarray(a, dtype=np.float32))
    b16 = lambda a: np.ascontiguousarray(
        np.asarray(a, dtype=np.float32).astype(ml_dtypes.bfloat16)
    )
    base = {
        "wq1t": b16(np.asarray(wq1).T),
        "wq2t": b16(np.asarray(wq2).T),
        "wkt": b16(np.asarray(wk).T),
        "wvt": b16(np.asarray(wv).T),
        "wo1t": b16(np.asarray(wo)[:, :E].T),
        "wo2t": b16(np.asarray(wo)[:, E:].T),
        "bq1": f32(bq1),
        "bq2": f32(bq2),
        "bk": f32(bk),
        "bv": f32(bv),
        "bo": f32(bo),
        "lnw": f32(ln_w),
        "lnb": f32(ln_b),
    }
    q1 = np.asarray(q1)
    q2 = np.asarray(q2)
    kv_flat = [b16(np.asarray(kv)[b].reshape(CKV, N)) for b in range(B)]
    in_maps = []
    for c in range(8):
        b, h = divmod(c, 2)
        m = dict(base)
        m["xq1"] = b16(q1[b, :, h * 32 : (h + 1) * 32, :].reshape(CQ, NQ))
        m["xq2"] = b16(q2[b, :, h * 32 : (h + 1) * 32, :].reshape(CQ, NQ))
        m["xkv"] = kv_flat[b]
        in_maps.append(m)
    return in_maps


def assemble_output(results):
    out = np.empty((B, E, 64, 64), dtype=np.float32)
    for c in range(8):
        b, h = divmod(c, 2)
        out[b, :, h * 32 : (h + 1) * 32, :] = results[c]["out"].reshape(E, 32, 64)
    return out


def kernel(**inputs):
    from concourse.bass_utils import run_bass_kernel_spmd

    nc = _get_nc()
    in_maps = make_in_maps(**inputs)
    res = run_bass_kernel_spmd(nc, in_maps, list(range(8)))
    return assemble_output(res.results)


if __name__ == "__main__":
    nc = build_nc()
    print("built ok")


# revision 26
# speedup vs baseline: 1.0862x; 1.0486x over previous
"""Trainium2 Bass kernel for nn_CrossAttention_79448305041860.

Dual cross-attention (q1, q2 vs shared kv) + concat + out-proj + LayerNorm,
B=4, E=256, N=64*64=4096 tokens.

Sharding: 8 cores = 4 batches x 2 query-token halves. Each core computes
K,V for its batch (replicated across the pair of cores sharing a batch) and
the full pipeline for its 2048-query-token slice. No cross-core comm.

v2 vs baseline:
  - All attention matmul operands are bf16 (inputs cast host-side, halving
    input DMA): same PE rate as fp32r but FWL weight loads (~2x faster,
    fully hidden) remove the ~9% LDWEIGHTS stall seen in the fp32r trace.
  - exp runs on [P, 1024] psum tiles (2 banks per S group) to amortize the
    ~352-cycle ACT fixed cost: 1.12 ns/elem vs 1.41 at 512.
  - Softmax-denominator accumulation in bf16 (2x DVE mode).
  - Phase 2 (out-proj + LN + transpose + store) is interleaved into the
    set-2 attention windows, one q-block behind, so its PE work fills the
    o_ps-drain bubbles and its DVE/ACT work hides under attention matmuls.
  - rstd = 1/sqrt(var+eps) via quake-rsqrt on DVE (bitcast + 2 Newton
    steps): keeps Sqrt out of the ACT queue, whose table set would thrash
    against exp (no ACT table set contains both).
"""

import numpy as np
from contextlib import ExitStack

import ml_dtypes

import concourse.bass as bass
import concourse.mybir as mybir
import concourse.tile as tile
from concourse import bacc
from concourse.masks import make_identity

FP32 = mybir.dt.float32
BF16 = mybir.dt.bfloat16
I32 = mybir.dt.int32
AF = mybir.ActivationFunctionType
ALU = mybir.AluOpType

P = 128
B = 4
E = 256            # embed dim
ET = E // P        # 2 e-tiles
CKV = 512          # kv channels
CT = CKV // P      # 4 c-tiles
CQ = 256           # q channels
CQT = CQ // P      # 2 c-tiles
N = 4096           # kv tokens per batch
NKT = N // P       # 32 k token-tiles
NK2 = NKT // 2     # 16 k-tile pairs
NQ = 2048          # query tokens per core
QB = 512           # q block (psum bank width)
NQB = NQ // QB     # 4 q blocks
NT = NQ // P       # 16 token-tiles per core
SCALE = 1.0 / 16.0  # 1/sqrt(E)
LN_EPS = 1e-5
QUAKE = 0x5F3759DF


def _bcast_row(nc, dram_handle, sbuf_tile):
    """DMA-broadcast a [E] dram vector to all partitions of a [P, E] tile."""
    src_ap = dram_handle[:]
    bcast = bass.AP(
        tensor=src_ap.tensor,
        offset=src_ap.offset,
        ap=[[0, P], *src_ap.ap],
    )
    nc.gpsimd.dma_start(out=sbuf_tile[:], in_=bcast)


def build_nc():
    nc = bacc.Bacc()

    xq1_d = nc.dram_tensor("xq1", [CQ, NQ], BF16, kind="ExternalInput")
    xq2_d = nc.dram_tensor("xq2", [CQ, NQ], BF16, kind="ExternalInput")
    xkv_d = nc.dram_tensor("xkv", [CKV, N], BF16, kind="ExternalInput")
    wq1t_d = nc.dram_tensor("wq1t", [CQ, E], BF16, kind="ExternalInput")
    wq2t_d = nc.dram_tensor("wq2t", [CQ, E], BF16, kind="ExternalInput")
    wkt_d = nc.dram_tensor("wkt", [CKV, E], BF16, kind="ExternalInput")
    wvt_d = nc.dram_tensor("wvt", [CKV, E], BF16, kind="ExternalInput")
    wo1t_d = nc.dram_tensor("wo1t", [E, E], BF16, kind="ExternalInput")
    wo2t_d = nc.dram_tensor("wo2t", [E, E], BF16, kind="ExternalInput")
    bq1_d = nc.dram_tensor("bq1", [E], FP32, kind="ExternalInput")
    bq2_d = nc.dram_tensor("bq2", [E], FP32, kind="ExternalInput")
    bk_d = nc.dram_tensor("bk", [E], FP32, kind="ExternalInput")
    bv_d = nc.dram_tensor("bv", [E], FP32, kind="ExternalInput")
    bo_d = nc.dram_tensor("bo", [E], FP32, kind="ExternalInput")
    lnw_d = nc.dram_tensor("lnw", [E], FP32, kind="ExternalInput")
    lnb_d = nc.dram_tensor("lnb", [E], FP32, kind="ExternalInput")
    out_d = nc.dram_tensor("out", [E, NQ], FP32, kind="ExternalOutput")

    with tile.TileContext(nc) as tc, ExitStack() as ctx:
        const = ctx.enter_context(tc.tile_pool(name="const", bufs=1))
        wts = ctx.enter_context(tc.tile_pool(name="wts", bufs=1))
        bigin = ctx.enter_context(tc.tile_pool(name="bigin", bufs=1))
        keep = ctx.enter_context(tc.tile_pool(name="keep", bufs=1))
        flow = ctx.enter_context(tc.tile_pool(name="flow", bufs=1))
        ps_s = ctx.enter_context(tc.tile_pool(name="ps_s", bufs=2, space="PSUM"))
        ps_o = ctx.enter_context(tc.tile_pool(name="ps_o", bufs=2, space="PSUM"))
        ps_c = ctx.enter_context(tc.tile_pool(name="ps_c", bufs=2, space="PSUM"))

        # ---- PE warmup ----
        # Dummy matmuls on a memset tile (no DMA dependency -- runs as soon
        # as the engine prologues finish) while the first kv chunks stream
        # in: keeps the PE_HAM activity window busy so the clock gate opens
        # to 2.4 GHz before the first real projection matmul (otherwise
        # phase 0 runs at the cold 1.2 GHz for its first ~20us).
        warm_w = const.tile([P, P], BF16, name="warm_w")
        nc.vector.memset(warm_w, 1.0)
        warm_ps = ps_s.tile([P, 2, QB], FP32, name="warm_ps", tag="s")
        for _ in range(40):
            nc.tensor.matmul(
                warm_ps[:, 0, :P], warm_w[:], warm_w[:], start=True, stop=True
            )

        # ---- weights / biases ----
        # The kv-path weights ride the two HWDGE queues ahead of the first
        # kv chunks so the first projection matmuls start ~2us earlier;
        # everything later goes on gpsimd (SWDGE).
        def _load_w(name, dram, ctiles, eng):
            t = wts.tile([P, ctiles, E], BF16, name=name)
            eng.dma_start(t[:], dram[:].rearrange("(o p) e -> p o e", p=P))
            return t

        wkt = _load_w("wkt", wkt_d, CT, nc.sync)
        wvt = _load_w("wvt", wvt_d, CT, nc.scalar)
        bk = wts.tile([P, ET], FP32, name="bk")
        nc.gpsimd.dma_start(bk[:], bk_d[:].rearrange("(o p) -> p o", p=P))
        bv_b = wts.tile([P, E], FP32, name="bv_b")
        _bcast_row(nc, bv_d, bv_b)

        wq1t = _load_w("wq1t", wq1t_d, CQT, nc.gpsimd)
        wq2t = _load_w("wq2t", wq2t_d, CQT, nc.gpsimd)
        bq1 = wts.tile([P, ET], FP32, name="bq1")
        nc.gpsimd.dma_start(bq1[:], bq1_d[:].rearrange("(o p) -> p o", p=P))
        bq2 = wts.tile([P, ET], FP32, name="bq2")
        nc.gpsimd.dma_start(bq2[:], bq2_d[:].rearrange("(o p) -> p o", p=P))

        wo1t = _load_w("wo1t", wo1t_d, ET, nc.gpsimd)
        wo2t = _load_w("wo2t", wo2t_d, ET, nc.gpsimd)
        bo_b = wts.tile([P, E], FP32, name="bo_b")
        _bcast_row(nc, bo_d, bo_b)
        # LN affine folded into the post-transpose ACT drain, where the
        # channel dim sits on partitions: per-partition scalar layout.
        lnw_c = wts.tile([P, ET], FP32, name="lnw_c")
        nc.gpsimd.dma_start(lnw_c[:], lnw_d[:].rearrange("(o p) -> p o", p=P))
        lnb_c = wts.tile([P, ET], FP32, name="lnb_c")
        nc.gpsimd.dma_start(lnb_c[:], lnb_d[:].rearrange("(o p) -> p o", p=P))

        # ---- constants ----
        ident = const.tile([P, P], FP32, name="ident")
        make_identity(nc, ident)
        ones = const.tile([P, 2], BF16, name="ones")
        nc.vector.memset(ones, 1.0)

        # ---- phase 0: K^T, V projections; Q^T streamed per window ----
        ktm = keep.tile([P, ET, N], BF16, name="ktm")    # K^T e-major
        vtm = keep.tile([P, NKT, E], BF16, name="vtm")   # V token-major

        QCH = 512
        qt1 = keep.tile([P, ET, NQ], BF16, name="qt1")   # Q1^T e-major
        qt2 = keep.tile([P, ET, NQ], BF16, name="qt2")
        q_specs = [
            (xq_d, wqt, bq, qt, ch)
            for (xq_d, wqt, bq, qt) in (
                (xq1_d, wq1t, bq1, qt1),
                (xq2_d, wq2t, bq2, qt2),
            )
            for ch in range(NQ // QCH)
        ]

        def _load_xq(spec):
            # q-input chunks ride the scalar HWDGE queue BEHIND the kv
            # chunks (emitted after the kv loop below), so kv streaming is
            # never delayed but the q inputs still arrive well before their
            # (mid-window-hoisted) projections.
            xq_d, _, _, _, ch = spec
            t = bigin.tile([P, CQT, QCH], BF16, name="xq", tag="xq", bufs=2)
            nc.scalar.dma_start(
                t[:],
                xq_d[:].rearrange("(o p) n -> p o n", p=P)[
                    :, :, ch * QCH : (ch + 1) * QCH
                ],
            )
            return t

        PREFETCH = 2
        xq_tiles = {}

        # kv streams in small chunks -- smallest first so the PE starts
        # within ~2us -- alternating between the two HWDGE queues
        # (sync / scalar) to double streaming bandwidth.
        KV_CHUNKS = [128, 128, 256, 256] + [256] * 13
        kv_off = 0
        for ci, kvch in enumerate(KV_CHUNKS):
            xkv_sb = bigin.tile([P, CT, 256], BF16, name="xkv", tag="xkv", bufs=4)
            dma_eng = nc.sync if ci % 2 == 0 else nc.scalar
            dma_eng.dma_start(
                xkv_sb[:, :, :kvch],
                xkv_d[:].rearrange("(o p) n -> p o n", p=P)[
                    :, :, kv_off : kv_off + kvch
                ],
            )
            # K^T for these token-columns
            for t in range(ET):
                for cc in range(0, kvch, QB):
                    w = min(QB, kvch - cc)
                    ps = ps_s.tile([P, 2, QB], FP32, name="kps", tag="s")
                    for j in range(CT):
                        nc.tensor.matmul(
                            ps[:, 0, :w],
                            wkt[:, j, t * P : (t + 1) * P],
                            xkv_sb[:, j, cc : cc + w],
                            start=(j == 0),
                            stop=(j == CT - 1),
                        )
                    nc.scalar.activation(
                        ktm[:, t, kv_off + cc : kv_off + cc + w],
                        ps[:, 0, :w],
                        AF.Identity,
                        bias=bk[:, t : t + 1],
                        scale=1.0,
                    )
            # V for these token-rows
            for v in range(kvch // P):
                kt_idx = (kv_off // P) + v
                ps = ps_o.tile([P, E], FP32, name="vps", tag="o")
                for j in range(CT):
                    nc.tensor.matmul(
                        ps[:],
                        xkv_sb[:, j, v * P : (v + 1) * P],
                        wvt[:, j, :],
                        start=(j == 0),
                        stop=(j == CT - 1),
                    )
                nc.vector.tensor_tensor(vtm[:, kt_idx, :], ps[:], bv_b[:], ALU.add)
            kv_off += kvch

        for i in range(PREFETCH):
            xq_tiles[i] = _load_xq(q_specs[i])

        def _qt_proj_chunk(i):
            """Project one streamed q-input chunk into its Q^T slice."""
            xq_d, wqt, bq, qt, ch = q_specs[i]
            xq_sb = xq_tiles.pop(i)
            if i + PREFETCH < len(q_specs):
                xq_tiles[i + PREFETCH] = _load_xq(q_specs[i + PREFETCH])
            for t in range(ET):
                ps = ps_s.tile([P, 2, QB], FP32, name="qps", tag="s")
                for j in range(CQT):
                    nc.tensor.matmul(
                        ps[:, 0, :],
                        wqt[:, j, t * P : (t + 1) * P],
                        xq_sb[:, j, :],
                        start=(j == 0),
                        stop=(j == CQT - 1),
                    )
                nc.vector.tensor_scalar(
                    qt[:, t, ch * QCH : (ch + 1) * QCH],
                    ps[:, 0, :],
                    bq[:, t : t + 1],
                    None,
                    op0=ALU.add,
                )

        # ---- phases 1+2: attention, with out-proj/LN/store interleaved ----
        o1ut = keep.tile([P, ET, NQ], BF16, name="o1ut")  # unnormalized out1^T
        o2ut = keep.tile([P, ET, NQ], BF16, name="o2ut")
        r1 = keep.tile([P, NT], FP32, name="r1")          # 1/denom per token
        r2 = keep.tile([P, NT], FP32, name="r2")

        out_r = out_d[:].rearrange("(o p) n -> p o n", p=P)
        bo_bc = bo_b[:, None, :].to_broadcast([P, 2, E])

        # per-qb phase-2 state, carried one window
        state = {}

        def _attention_kloop(si, qb, qt):
            """S -> exp -> PV for one (set, q-block). The S matmuls for pair
            k2+1 are issued BEFORE the PV matmuls for pair k2 so the exp
            latency hides under PE work (engines execute their queues
            in-order)."""
            if si == 0 and qb == 0:
                # later windows' q-projections are hoisted into the middle
                # of the previous window (see _attention_rest)
                _qt_proj_chunk(0)
            qsl = slice(qb * QB, (qb + 1) * QB)
            o_ps = [
                ps_o.tile([P, QB], FP32, name=f"ops{t}", tag="o")
                for t in range(ET)
            ]
            acc2 = flow.tile([P, 2, QB], BF16, name="acc2", tag="acc", bufs=2)
            s_tiles = {}

            def emit_s(k2):
                s_ps = ps_s.tile([P, 2, QB], FP32, name="sps", tag="s")
                for kk in range(2):
                    k = 2 * k2 + kk
                    for t in range(ET):
                        nc.tensor.matmul(
                            s_ps[:, kk, :],
                            ktm[:, t, k * P : (k + 1) * P],
                            qt[:, t, qsl],
                            start=(t == 0),
                            stop=(t == ET - 1),
                        )
                s_tiles[k2] = s_ps

            emit_s(0)
            emit_s(1)
            return qsl, o_ps, acc2, s_tiles

        def _attention_rest(si, qb, qt, qsl, o_ps, acc2, s_tiles, out_t):
            w = si * NQB + qb
            for k2 in range(NK2):
                if k2 == NK2 // 2 and w + 1 < 2 * NQB:
                    # hoist the NEXT window's q-projection to mid-window so
                    # its psum drain is long done before that window's first
                    # S matmul (removes a DVE wait at every boundary)
                    _qt_proj_chunk(w + 1)
                s_ps = s_tiles.pop(k2)
                pt = flow.tile([P, 2, QB], BF16, name="pt", tag="pt", bufs=8)
                nc.scalar.activation(pt[:], s_ps[:], AF.Exp, scale=SCALE)
                for kk in range(2):
                    k = 2 * k2 + kk
                    for t in range(ET):
                        nc.tensor.matmul(
                            o_ps[t][:],
                            vtm[:, k, t * P : (t + 1) * P],
                            pt[:, kk, :],
                            start=(k2 == 0 and kk == 0),
                            stop=(k2 == NK2 - 1 and kk == 1),
                        )
                if k2 + 2 < NK2:
                    # keep the S stream one pair ahead of PV
                    s_ps2 = ps_s.tile([P, 2, QB], FP32, name="sps", tag="s")
                    for kk in range(2):
                        k = 2 * (k2 + 2) + kk
                        for t in range(ET):
                            nc.tensor.matmul(
                                s_ps2[:, kk, :],
                                ktm[:, t, k * P : (k + 1) * P],
                                qt[:, t, qsl],
                                start=(t == 0),
                                stop=(t == ET - 1),
                            )
                    s_tiles[k2 + 2] = s_ps2
                if k2 == 0:
                    nc.vector.tensor_copy(acc2[:], pt[:])
                else:
                    nc.vector.tensor_tensor(acc2[:], acc2[:], pt[:], ALU.add)
            # out^T psum drains (gate the next window's PV via o_ps rotation)
            # and the bf16 accumulator merge for the denominators.
            for t in range(ET):
                nc.vector.tensor_copy(out_t[:, t, qsl], o_ps[t][:])
            acc = flow.tile([P, QB], BF16, name="acc", tag="accm", bufs=2)
            nc.vector.tensor_tensor(acc[:], acc2[:, 0, :], acc2[:, 1, :], ALU.add)
            return acc

        def _attention_denom(qb, acc, r_t):
            """Denominator matmuls + reciprocal; for set-1 this is deferred
            into the NEXT window's head so the PE never waits on the DVE
            accumulator merge."""
            d_ps = ps_c.tile([P, QB // P, 2], FP32, name="dps", tag="c")
            for i in range(QB // P):
                nc.tensor.matmul(
                    d_ps[:, i, :],
                    acc[:, i * P : (i + 1) * P],
                    ones[:],
                    start=True,
                    stop=True,
                )
            nc.vector.reciprocal(
                r_t[:, qb * (QB // P) : (qb + 1) * (QB // P)], d_ps[:, :, 0]
            )

        def _quake_rstd(var_ap, n):
            """rstd = 1/sqrt(var+eps) on DVE: quake initial guess + 2 Newton
            steps (keeps Sqrt out of the ACT queue -- table-set thrash)."""
            vr = flow.tile([P, n], FP32, name="vr", tag="vr", bufs=2)
            yi = flow.tile([P, n], I32, name="yi", tag="yi", bufs=2)
            t1 = flow.tile([P, n], FP32, name="t1", tag="t1", bufs=2)
            rstd = flow.tile([P, n], FP32, name="rstd", tag="rstd", bufs=4)
            nc.vector.tensor_scalar(vr[:], var_ap, LN_EPS, None, op0=ALU.add)
            nc.vector.tensor_scalar(
                yi[:], vr[:].bitcast(I32), 1, None, op0=ALU.logical_shift_right
            )
            nc.vector.tensor_scalar(yi[:], yi[:], -1, None, op0=ALU.bitwise_xor)
            nc.vector.tensor_scalar(yi[:], yi[:], QUAKE + 1, None, op0=ALU.add)
            y0 = yi[:].bitcast(FP32)
            nc.vector.tensor_tensor(t1[:], y0, y0, ALU.mult)
            nc.vector.tensor_tensor(t1[:], t1[:], vr[:], ALU.mult)
            nc.vector.tensor_scalar(t1[:], t1[:], -0.5, 1.5, op0=ALU.mult, op1=ALU.add)
            nc.vector.tensor_tensor(rstd[:], y0, t1[:], ALU.mult)
            nc.vector.tensor_tensor(t1[:], rstd[:], rstd[:], ALU.mult)
            nc.vector.tensor_tensor(t1[:], t1[:], vr[:], ALU.mult)
            nc.vector.tensor_scalar(t1[:], t1[:], -0.5, 1.5, op0=ALU.mult, op1=ALU.add)
            nc.vector.tensor_tensor(rstd[:], rstd[:], t1[:], ALU.mult)
            return rstd

        def _outproj_pair(qb, pr, y):
            """Out-proj + softmax-normalize + combine for one token-pair."""
            for h in range(2):
                nt = qb * 4 + pr * 2 + h
                nsl = slice(nt * P, (nt + 1) * P)
                yp = ps_c.tile([P, 2, E], FP32, name="yp", tag="c")
                for j in range(ET):
                    nc.tensor.matmul(
                        yp[:, 0, :],
                        o1ut[:, j, nsl],
                        wo1t[:, j, :],
                        start=(j == 0),
                        stop=(j == ET - 1),
                    )
                for j in range(ET):
                    nc.tensor.matmul(
                        yp[:, 1, :],
                        o2ut[:, j, nsl],
                        wo2t[:, j, :],
                        start=(j == 0),
                        stop=(j == ET - 1),
                    )
                # y = y1*r1 (ACT) ; y += y2*r2 (DVE, fused)
                nc.scalar.activation(
                    y[:, h, :], yp[:, 0, :], AF.Identity,
                    scale=r1[:, nt : nt + 1],
                )
                nc.vector.scalar_tensor_tensor(
                    y[:, h, :], yp[:, 1, :], r2[:, nt : nt + 1], y[:, h, :],
                    op0=ALU.mult, op1=ALU.add,
                )
            nc.vector.tensor_tensor(y[:], y[:], bo_bc, ALU.add)

        def _transpose_store_tile(nt, y_h):
            """Transpose one token-tile to channel-major, LN-affine, store."""
            tp = ps_c.tile([P, ET, P], FP32, name="tp", tag="c")
            yt = flow.tile([P, ET, P], FP32, name="yt", tag="yt", bufs=3)
            for t in range(ET):
                nc.tensor.transpose(
                    tp[:, t, :], y_h[:, t * P : (t + 1) * P], ident[:]
                )
            for t in range(ET):
                nc.scalar.activation(
                    yt[:, t, :], tp[:, t, :], AF.Identity,
                    bias=lnb_c[:, t : t + 1], scale=lnw_c[:, t : t + 1],
                )
            nsl = slice(nt * P, (nt + 1) * P)
            for t in range(ET):
                nc.sync.dma_start(out_r[:, t, nsl], yt[:, t, :])

        def _phase2_front(qb):
            """Out-proj + softmax-normalize + combine + LN stats for the 4
            token-tiles of set-2 q-block qb. Emitted right after qb's
            attention window."""
            ys = []
            mv = flow.tile([P, 4, 2], FP32, name="mv", tag="mv", bufs=2)
            for pr in range(2):
                y = flow.tile([P, 2, E], FP32, name="y", tag="y", bufs=4)
                ys.append(y)
                _outproj_pair(qb, pr, y)
                for h in range(2):
                    st6 = flow.tile([P, 6], FP32, name="st6", tag="st6", bufs=3)
                    nc.vector.bn_stats(out=st6[:], in_=y[:, h, :])
                    nc.vector.bn_aggr(out=mv[:, pr * 2 + h, :], in_=st6[:])
            rstd = _quake_rstd(mv[:, :, 1], 4)
            state[qb] = (ys, mv, rstd)

        def _phase2_back_dve(qb):
            """Normalize for q-block qb (window qb+1, early). The LN affine
            rides the post-transpose ACT drain instead."""
            ys, mv, rstd = state[qb]
            for pr in range(2):
                y = ys[pr]
                for h in range(2):
                    i = pr * 2 + h
                    nc.vector.tensor_scalar(
                        y[:, h, :], y[:, h, :],
                        mv[:, i, 0:1], rstd[:, i : i + 1],
                        op0=ALU.subtract, op1=ALU.mult,
                    )

        def _phase2_back_pe(qb):
            """Transpose to channel-major + affine + store for q-block qb."""
            ys, _, _ = state.pop(qb)
            for pr in range(2):
                for h in range(2):
                    _transpose_store_tile(qb * 4 + pr * 2 + h, ys[pr][:, h, :])

        # set 1: plain attention windows. The denominator matmuls of window
        # qb run inside window qb+1's head so the PE never waits on the DVE
        # accumulator merge at a window boundary.
        pend = None
        for qb in range(NQB):
            qsl, o_ps, acc2, s_tiles = _attention_kloop(0, qb, qt1)
            if pend is not None:
                _attention_denom(*pend)
            acc = _attention_rest(0, qb, qt1, qsl, o_ps, acc2, s_tiles, o1ut)
            pend = (qb, acc, r1)
        # set 2: attention + interleaved phase-2 (one q-block behind)
        for qb in range(NQB):
            if qb > 0:
                _phase2_back_dve(qb - 1)
            qsl, o_ps, acc2, s_tiles = _attention_kloop(1, qb, qt2)
            if pend is not None:
                _attention_denom(*pend)
                pend = None
            acc = _attention_rest(1, qb, qt2, qsl, o_ps, acc2, s_tiles, o2ut)
            if qb > 0:
                _phase2_back_pe(qb - 1)
            _attention_denom(qb, acc, r2)
            _phase2_front(qb)
        _phase2_back_dve(NQB - 1)
        _phase2_back_pe(NQB - 1)

    nc.compile()
    return nc


_CACHE = {}


def _get_nc():
    if "nc" not in _CACHE:
        _CACHE["nc"] = build_nc()
    return _CACHE["nc"]


def make_in_maps(q1, q2, kv, wq1, bq1, wq2, bq2, wk, bk, wv, bv, wo, bo, ln_w, ln_b):
    f32 = lambda a: np.ascontiguousarray(np.asarray(a, dtype=np.float32))
    b16 = lambda a: np.ascontiguousarray(
        np.asarray(a, dtype=np.float32).astype(ml_dtypes.bfloat16)
    )
    base = {
        "wq1t": b16(np.asarray(wq1).T),
        "wq2t": b16(np.asarray(wq2).T),
        "wkt": b16(np.asarray(wk).T),
        "wvt": b16(np.asarray(wv).T),
        "wo1t": b16(np.asarray(wo)[:, :E].T),
        "wo2t": b16(np.asarray(wo)[:, E:].T),
        "bq1": f32(bq1),
        "bq2": f32(bq2),
        "bk": f32(bk),
        "bv": f32(bv),
        "bo": f32(bo),
        "lnw": f32(ln_w),
        "lnb": f32(ln_b),
    }
    q1 = np.asarray(q1)
    q2 = np.asarray(q2)
    kv_flat = [b16(np.asarray(kv)[b].reshape(CKV, N)) for b in range(B)]
    in_maps = []
    for c in range(8):
        b, h = divmod(c, 2)
        m = dict(base)
        m["xq1"] = b16(q1[b, :, h * 32 : (h + 1) * 32, :].reshape(CQ, NQ))
        m["xq2"] = b16(q2[b, :, h * 32 : (h + 1) * 32, :].reshape(CQ, NQ))
        m["xkv"] = kv_flat[b]
        in_maps.append(m)
    return in_maps


def assemble_output(results):
    out = np.empty((B, E, 64, 64), dtype=np.float32)
    for c in range(8):
        b, h = divmod(c, 2)
        out[b, :, h * 32 : (h + 1) * 32, :] = results[c]["out"].reshape(E, 32, 64)
    return out


def kernel(**inputs):
    from concourse.bass_utils import run_bass_kernel_spmd

    nc = _get_nc()
    in_maps = make_in_maps(**inputs)
    res = run_bass_kernel_spmd(nc, in_maps, list(range(8)))
    return assemble_output(res.results)


if __name__ == "__main__":
    nc = build_nc()
    print("built ok")
